# revision 29
# baseline (speedup 1.0000x reference)
"""BitNetLinear (ternary eval-mode) forward on 8 trn2 NeuronCores.

Math (reference):
    s_w  = max(mean|W|, eps);  q = sign(W) * (|W/s_w| > 0.5)
    s_x  = max(mean|x|, eps)
    out  = (x/s_x) @ (q*s_w)^T * s_x + bias * s_x
         = x @ q^T * s_w + bias * s_x          (exact in real arithmetic)

Sharding: 2D grid, TG=4 token groups x FG=2 out-feature groups.
Each core: T=1024 tokens, O=2048 out features, I=4096 contraction.
Host passes x and W shards PRE-TRANSPOSED (i-major) so both matmul
operands already have the contraction dim on partitions — no on-chip
transposes. s_w needs a global view of W: each core reduces |.| over a
distinct 1/8 of W and a 1-scalar AllReduce(add) produces the global
sum. bias*s_x is added on the host (bias is identically zero for this
problem; host uses the exact reference formula).

Device pipeline per core:
  - |W| partial sum over its eighth (DVE abs-reduce + GPSIMD C-reduce)
  - AllReduce scalar -> s_w, thr = 0.5*s_w on chip
  - x^T strips (bf16, host-cast): DMA into resident tiles [128i, T]
  - per 512-wide o-chunk, per i-block: DMA w^T strip [128i, 512o],
    quantize to 2q in {-2,0,2} bf16 via
        t2 = (w > thr) * 2          (DVE tensor_scalar, fused dual op)
        s2 = Sign(w + thr)          (ACT activation)
        q2 = (t2 - 1) + s2          (DVE scalar_tensor_tensor)
    then matmul sweep: psum[t,o] += xT.T @ q2T (fp32 PSUM, K=4096)
    and evict with scale thr (= s_w/2, undoing the 2x) on ACT.
"""

import sys

sys.path.insert(0, "/opt/trn_rl_repo")

import numpy as np

P = 128
EPS = 1e-8
# Recursive-doubling remote-SDMA all-reduce: validated in MultiCoreSim but
# the InstRemoteDMABroadcastDescs path fails on this runtime (INTERNAL error
# at execute) — keep the ncfw collective.
USE_REMOTE_EXCHANGE = False

B, S = 2, 2048
I_FULL = 4096  # in_features
O_FULL = 4096  # out_features
N_CORES = 8
TG, FG = 4, 2
T_SH = (B * S) // TG  # 1024
O_SH = O_FULL // FG  # 2048


def build_nc(T, O, I, n_cores, tg, w_elems_total):
    """Build + compile the SPMD Bass module for one core shape."""
    from concourse import bacc, mybir, tile
    import concourse.bass as bass
    from concourse.bass import ts, ds

    f32 = mybir.dt.float32
    bf16 = mybir.dt.bfloat16
    A = mybir.AluOpType

    assert T % P == 0 and O % P == 0 and I % P == 0

    nc = bacc.Bacc(
        "TRN2", target_bir_lowering=False, debug=False, num_devices=n_cores
    )
    # all inputs pre-transposed on host: i-major; x pre-cast to bf16
    xT = nc.dram_tensor("xT", [I, T], bf16, kind="ExternalInput").ap()
    wT = nc.dram_tensor("wT", [I, O], f32, kind="ExternalInput").ap()
    out_sh = nc.dram_tensor("out_sh", [T, O], f32, kind="ExternalOutput").ap()

    n_tb = T // P
    n_ib = I // P
    OC = min(512, O)  # o-chunk width
    n_oc = O // OC
    i_slab = I // tg  # rows of wT this core abs-sums

    with tile.TileContext(nc) as tc:
        with (
            tc.tile_pool(name="scal", bufs=1) as scal_pool,
            tc.tile_pool(name="dram", bufs=1, space="DRAM") as dram_pool,
            tc.tile_pool(name="sumw", bufs=4) as sum_pool,
            tc.tile_pool(name="xt", bufs=1) as xt_pool,
            tc.tile_pool(name="win", bufs=10) as win_pool,
            tc.tile_pool(name="tq", bufs=6) as tq_pool,
            tc.tile_pool(name="sq", bufs=6) as sq_pool,
            tc.tile_pool(name="qt", bufs=1) as qt_pool,
            tc.tile_pool(name="osb", bufs=6) as out_pool,
            tc.tile_pool(name="psacc", bufs=1, space="PSUM") as ps_acc,
        ):
            # ---- phase S: partial sum of |W| over this core's i-slab.
            # The host rotates wT's i-rows per core so rows [0, i_slab)
            # are this core's distinct slab (see make_in_maps). Half-width
            # strips keep the DMAs fine-grained so they interleave with the
            # x/w prefetch instead of head-of-line blocking it.
            OH = O // 2
            n_sum = 2 * (i_slab // P)
            acc = scal_pool.tile([P, n_sum], f32)
            for r in range(n_sum):
                wst = sum_pool.tile([P, OH], f32, tag="ws")
                nc.sync.dma_start(
                    wst[:], wT[ts(r // 2, P), ds((r % 2) * OH, OH)]
                )
                nc.vector.tensor_reduce(
                    acc[:, r : r + 1],
                    wst[:],
                    axis=mybir.AxisListType.X,
                    op=A.add,
                    apply_absolute_value=True,
                )
            red = scal_pool.tile([P, 1], f32)
            nc.vector.tensor_reduce(
                red[:], acc[:], axis=mybir.AxisListType.X, op=A.add
            )

            if USE_REMOTE_EXCHANGE and n_cores == 8:
                # ---- phase C': recursive-doubling all-reduce of the
                # [128,1] partials via pairwise remote SDMA (XOR-relative
                # dests keep the program SPMD-uniform). Avoids the ncfw
                # collective's ~40us init barrier + ~13us latency. The
                # reduction tree is symmetric, so every core computes a
                # bitwise-identical sum.
                ex_sems = [nc.alloc_semaphore(f"ex_arrive{r}") for r in range(3)]
                ls_sem = nc.alloc_semaphore("ex_sent")
                bufs = [
                    scal_pool.tile([P, 1], f32, name=f"exbuf{r}") for r in range(3)
                ]
                acc_r = red
                for r, step in enumerate((1, 2, 4)):
                    rdests = [None] * 8
                    slot = 4 if step == 4 else 0
                    rdests[slot] = (0, step)
                    with tc.tile_critical():
                        nc.gpsimd.remote_dma_broadcast(
                            bufs[r][:],
                            acc_r[:],
                            remote_sem=ex_sems[r],
                            local_sem=ls_sem,
                            rdests=rdests,
                        )
                        nc.gpsimd.trigger_dma(count=None)
                    nxt = scal_pool.tile([P, 1], f32, name=f"excum{r}")
                    with tc.tile_critical():
                        nc.vector.tensor_tensor(
                            out=nxt[:], in0=acc_r[:], in1=bufs[r][:], op=A.add
                        )._wait_ge(ex_sems[r], 2)
                    acc_r = nxt
                sb_s = scal_pool.tile([1, 1], f32)
                nc.gpsimd.tensor_reduce(
                    sb_s[:], acc_r[:], axis=mybir.AxisListType.C, op=A.add
                )
                s_sum = scal_pool.tile([P, 1], f32)
                nc.gpsimd.partition_broadcast(s_sum[:], sb_s[:])
            else:
                sb_s = scal_pool.tile([1, 1], f32)
                nc.gpsimd.tensor_reduce(
                    sb_s[:], red[:], axis=mybir.AxisListType.C, op=A.add
                )
                # ---- phase C: AllReduce the scalar across all cores ----
                cc_in = dram_pool.tile([1, 1], f32)
                cc_out = dram_pool.tile([1, 1], f32)
                nc.sync.dma_start(cc_in[:], sb_s[:])
                nc.gpsimd.collective_compute(
                    "AllReduce",
                    A.add,
                    replica_groups=[list(range(n_cores))],
                    ins=[cc_in[:]],
                    outs=[cc_out[:]],
                )
                cc_out_ap = cc_out[:]
                bcast_ap = bass.AP(
                    tensor=cc_out_ap.tensor,
                    offset=cc_out_ap.offset,
                    ap=[[0, P], [1, 1]],
                )
                s_sum = scal_pool.tile([P, 1], f32)
                nc.sync.dma_start(s_sum[:], bcast_ap)
            sw = scal_pool.tile([P, 1], f32)
            nc.vector.tensor_scalar(
                out=sw[:],
                in0=s_sum[:],
                scalar1=1.0 / float(w_elems_total),
                scalar2=EPS,
                op0=A.mult,
                op1=A.max,
            )
            thr = scal_pool.tile([P, 1], f32)
            nc.vector.tensor_scalar(
                out=thr[:], in0=sw[:], scalar1=0.5, scalar2=None, op0=A.mult
            )

            # ---- quantize helper: w^T strip [128i, OC] -> 2q in bf16 ----
            def quantize(c, ib):
                wst = win_pool.tile([P, OC], f32, tag="w", name=f"w_{c}_{ib}")
                nc.sync.dma_start(wst[:], wT[ts(ib, P), ds(c * OC, OC)])
                t2 = tq_pool.tile([P, OC], bf16, tag="t2", name=f"t2_{c}_{ib}")
                nc.vector.tensor_scalar(
                    out=t2[:],
                    in0=wst[:],
                    scalar1=thr[:],
                    scalar2=2.0,
                    op0=A.is_gt,
                    op1=A.mult,
                )
                s2 = sq_pool.tile([P, OC], bf16, tag="s2", name=f"s2_{c}_{ib}")
                nc.scalar.activation(
                    s2[:], wst[:], mybir.ActivationFunctionType.Sign, bias=thr[:]
                )
                q2 = qt_pool.tile(
                    [P, OC], bf16, tag=f"qt_{ib}_{c % 2}", name=f"qt_{c}_{ib}"
                )
                # q2 = (t2 - 1) + s2  in {-2, 0, 2}  (= 2q)
                nc.vector.scalar_tensor_tensor(
                    out=q2[:],
                    in0=t2[:],
                    scalar=-1.0,
                    in1=s2[:],
                    op0=A.add,
                    op1=A.add,
                )
                return q2

            psk = [0]  # rotating PSUM tag counter (8 banks)

            def evict(ps, c, tb):
                osb = out_pool.tile([P, OC], f32, tag="o")
                # psum holds x @ (2q)^T; scale by thr = s_w/2
                nc.scalar.activation(
                    osb[:], ps[:], mybir.ActivationFunctionType.Copy, scale=thr[:]
                )
                nc.sync.dma_start(out_sh[ts(tb, P), ds(c * OC, OC)], osb[:])

            def psum_tile(name):
                t = ps_acc.tile([P, OC], f32, tag=f"acc{psk[0] % 8}", name=name)
                psk[0] += 1
                return t

            # ---- chunk 0: i-block-major so matmuls start while x/w
            # stream in (PE never waits for the full first sweep).
            # Chunk 1 is quantized in the same pass.
            xt_tiles = [None] * n_ib
            qt_c = {}
            ps0 = [psum_tile(f"ps0_{tb}") for tb in range(n_tb)]
            for ib in range(n_ib):
                xb = xt_pool.tile([P, T], bf16, tag=f"xt_{ib}", name=f"xt_{ib}")
                nc.sync.dma_start(xb[:], xT[ts(ib, P), :])
                xt_tiles[ib] = xb
                qt_c[(0, ib)] = quantize(0, ib)
                for tb in range(n_tb):
                    nc.tensor.matmul(
                        ps0[tb][:],
                        lhsT=xt_tiles[ib][:, ts(tb, P)],
                        rhs=qt_c[(0, ib)][:],
                        start=(ib == 0),
                        stop=(ib == n_ib - 1),
                    )
            for tb in range(n_tb):
                evict(ps0[tb], 0, tb)

            # ---- remaining chunks: chunk 1 solo, then pairs sharing
            # one stationary block per (tb, ib).
            c = 1
            while c < n_oc:
                pair = [c] if (n_oc - c) % 2 == 1 else [c, c + 1]
                for cc in pair:
                    for ib in range(n_ib):
                        if (cc, ib) not in qt_c:
                            qt_c[(cc, ib)] = quantize(cc, ib)
                for tb in range(n_tb):
                    ps_tiles = [psum_tile(f"ps_{cc}_{tb}") for cc in pair]
                    for ib in range(n_ib):
                        lhs = xt_tiles[ib][:, ts(tb, P)]
                        for h, cc in enumerate(pair):
                            nc.tensor.matmul(
                                ps_tiles[h][:],
                                lhsT=lhs,
                                rhs=qt_c[(cc, ib)][:],
                                start=(ib == 0),
                                stop=(ib == n_ib - 1),
                            )
                    for h, cc in enumerate(pair):
                        evict(ps_tiles[h], cc, tb)
                for cc in pair:
                    for ib in range(n_ib):
                        del qt_c[(cc, ib)]
                c += len(pair)

    nc.compile()
    return nc


_CACHE = {}


def _get_nc(key):
    if key not in _CACHE:
        _CACHE[key] = build_nc(*key)
    return _CACHE[key]


def make_in_maps(x2d, weight, n_cores=N_CORES, tg=TG, fg=FG):
    """Host-side sharding: per-core pre-transposed inputs, x in bf16."""
    import ml_dtypes

    t_tot, i_full = x2d.shape
    o_full = weight.shape[0]
    t_sh = t_tot // tg
    o_sh = o_full // fg
    i_slab = i_full // tg
    x_bf = x2d.astype(ml_dtypes.bfloat16)
    wT_halves = {}
    for b in range(fg):
        wT_halves[b] = np.ascontiguousarray(weight[b * o_sh : (b + 1) * o_sh].T)
    in_maps = []
    for cid in range(n_cores):
        g, b = cid // fg, cid % fg
        # rotate i-rows of wT so rows [0, i_slab) are this core's slab;
        # the matmul contraction is a sum over i, invariant to the
        # rotation as long as xT rows are rotated identically.
        roll = -g * i_slab
        in_maps.append(
            {
                "xT": np.ascontiguousarray(
                    np.roll(x_bf[g * t_sh : (g + 1) * t_sh].T, roll, axis=0)
                ),
                "wT": np.roll(wT_halves[b], roll, axis=0),
            }
        )
    return in_maps


def run(x2d, weight, n_cores=N_CORES, tg=TG, fg=FG):
    """Run the sharded device computation: returns x @ q^T * s_w, [Ttot, O_full]."""
    from concourse.bass_utils import run_bass_kernel_spmd

    t_tot, i_full = x2d.shape
    o_full = weight.shape[0]
    t_sh = t_tot // tg
    o_sh = o_full // fg
    key = (t_sh, o_sh, i_full, n_cores, tg, o_full * i_full)
    nc = _get_nc(key)

    in_maps = make_in_maps(x2d, weight, n_cores, tg, fg)
    res = run_bass_kernel_spmd(nc, in_maps, core_ids=list(range(n_cores)))
    out = np.empty((t_tot, o_full), np.float32)
    for cid in range(n_cores):
        g, b = cid // fg, cid % fg
        out[g * t_sh : (g + 1) * t_sh, b * o_sh : (b + 1) * o_sh] = res.results[
            cid
        ]["out_sh"]
    return out


def kernel(x, weight, bias):
    x = np.asarray(x, np.float32)
    weight = np.asarray(weight, np.float32)
    bias = np.asarray(bias, np.float32)
    t_tot = x.shape[0] * x.shape[1]
    out = run(x.reshape(t_tot, x.shape[2]), weight)
    # bias term: out += bias * s_x (exact reference semantics; zero for
    # this problem's bias). The matmul term is s_x-invariant.
    if np.any(bias):
        s_x = np.float32(max(np.mean(np.abs(x)), EPS))
        out = out + (bias * s_x)[None, :]
    return out.reshape(x.shape[0], x.shape[1], weight.shape[0])


# revision 30
# speedup vs baseline: 1.0092x; 1.0092x over previous
"""BitNetLinear (ternary eval-mode) forward on 8 trn2 NeuronCores.

Math (reference):
    s_w  = max(mean|W|, eps);  q = sign(W) * (|W/s_w| > 0.5)
    s_x  = max(mean|x|, eps)
    out  = (x/s_x) @ (q*s_w)^T * s_x + bias * s_x
         = x @ q^T * s_w + bias * s_x          (exact in real arithmetic)

Sharding: 2D grid, TG=4 token groups x FG=2 out-feature groups.
Each core: T=1024 tokens, O=2048 out features, I=4096 contraction.
Host passes x and W shards PRE-TRANSPOSED (i-major) so both matmul
operands already have the contraction dim on partitions — no on-chip
transposes. s_w needs a global view of W: each core reduces |.| over a
distinct 1/8 of W and a 1-scalar AllReduce(add) produces the global
sum. bias*s_x is added on the host (bias is identically zero for this
problem; host uses the exact reference formula).

Device pipeline per core:
  - |W| partial sum over its eighth (DVE abs-reduce + GPSIMD C-reduce)
  - AllReduce scalar -> s_w, thr = 0.5*s_w on chip
  - x^T strips (bf16, host-cast): DMA into resident tiles [128i, T]
  - per 512-wide o-chunk, per i-block: DMA w^T strip [128i, 512o],
    quantize to 2q in {-2,0,2} bf16 via
        t2 = (w > thr) * 2          (DVE tensor_scalar, fused dual op)
        s2 = Sign(w + thr)          (ACT activation)
        q2 = (t2 - 1) + s2          (DVE scalar_tensor_tensor)
    then matmul sweep: psum[t,o] += xT.T @ q2T (fp32 PSUM, K=4096)
    and evict with scale thr (= s_w/2, undoing the 2x) on ACT.
"""

import sys

sys.path.insert(0, "/opt/trn_rl_repo")

import numpy as np

P = 128
EPS = 1e-8
# Recursive-doubling remote-SDMA all-reduce: validated in MultiCoreSim but
# the InstRemoteDMABroadcastDescs path fails on this runtime (INTERNAL error
# at execute) — keep the ncfw collective.
USE_REMOTE_EXCHANGE = False

B, S = 2, 2048
I_FULL = 4096  # in_features
O_FULL = 4096  # out_features
N_CORES = 8
TG, FG = 4, 2
T_SH = (B * S) // TG  # 1024
O_SH = O_FULL // FG  # 2048


def build_nc(T, O, I, n_cores, tg, w_elems_total):
    """Build + compile the SPMD Bass module for one core shape."""
    from concourse import bacc, mybir, tile
    import concourse.bass as bass
    from concourse.bass import ts, ds

    f32 = mybir.dt.float32
    bf16 = mybir.dt.bfloat16
    A = mybir.AluOpType

    assert T % P == 0 and O % P == 0 and I % P == 0

    nc = bacc.Bacc(
        "TRN2", target_bir_lowering=False, debug=False, num_devices=n_cores
    )
    # all inputs pre-transposed on host: i-major; x pre-cast to bf16
    xT = nc.dram_tensor("xT", [I, T], bf16, kind="ExternalInput").ap()
    wT = nc.dram_tensor("wT", [I, O], f32, kind="ExternalInput").ap()
    out_sh = nc.dram_tensor("out_sh", [T, O], f32, kind="ExternalOutput").ap()

    n_tb = T // P
    n_ib = I // P
    OC = min(512, O)  # o-chunk width
    n_oc = O // OC
    i_slab = I // tg  # rows of wT this core abs-sums

    with tile.TileContext(nc) as tc:
        with (
            tc.tile_pool(name="scal", bufs=1) as scal_pool,
            tc.tile_pool(name="dram", bufs=1, space="DRAM") as dram_pool,
            tc.tile_pool(name="sumw", bufs=4) as sum_pool,
            tc.tile_pool(name="xt", bufs=1) as xt_pool,
            tc.tile_pool(name="win", bufs=10) as win_pool,
            tc.tile_pool(name="tq", bufs=6) as tq_pool,
            tc.tile_pool(name="sq", bufs=6) as sq_pool,
            tc.tile_pool(name="qt", bufs=1) as qt_pool,
            tc.tile_pool(name="osb", bufs=6) as out_pool,
            tc.tile_pool(name="psacc", bufs=1, space="PSUM") as ps_acc,
        ):
            # ---- phase S: partial sum of |W| over this core's i-slab.
            # The host rotates wT's i-rows per core so rows [0, i_slab)
            # are this core's distinct slab (see make_in_maps). Half-width
            # strips keep the DMAs fine-grained so they interleave with the
            # x/w prefetch instead of head-of-line blocking it.
            OH = O // 2
            n_sum = 2 * (i_slab // P)
            acc = scal_pool.tile([P, n_sum], f32)
            for r in range(n_sum):
                wst = sum_pool.tile([P, OH], f32, tag="ws")
                nc.sync.dma_start(
                    wst[:], wT[ts(r // 2, P), ds((r % 2) * OH, OH)]
                )
                nc.vector.tensor_reduce(
                    acc[:, r : r + 1],
                    wst[:],
                    axis=mybir.AxisListType.X,
                    op=A.add,
                    apply_absolute_value=True,
                )
            red = scal_pool.tile([P, 1], f32)
            nc.vector.tensor_reduce(
                red[:], acc[:], axis=mybir.AxisListType.X, op=A.add
            )

            if USE_REMOTE_EXCHANGE and n_cores == 8:
                # ---- phase C': recursive-doubling all-reduce of the
                # [128,1] partials via pairwise remote SDMA (XOR-relative
                # dests keep the program SPMD-uniform). Avoids the ncfw
                # collective's ~40us init barrier + ~13us latency. The
                # reduction tree is symmetric, so every core computes a
                # bitwise-identical sum.
                ex_sems = [nc.alloc_semaphore(f"ex_arrive{r}") for r in range(3)]
                ls_sem = nc.alloc_semaphore("ex_sent")
                bufs = [
                    scal_pool.tile([P, 1], f32, name=f"exbuf{r}") for r in range(3)
                ]
                acc_r = red
                for r, step in enumerate((1, 2, 4)):
                    rdests = [None] * 8
                    slot = 4 if step == 4 else 0
                    rdests[slot] = (0, step)
                    with tc.tile_critical():
                        nc.gpsimd.remote_dma_broadcast(
                            bufs[r][:],
                            acc_r[:],
                            remote_sem=ex_sems[r],
                            local_sem=ls_sem,
                            rdests=rdests,
                        )
                        nc.gpsimd.trigger_dma(count=None)
                    nxt = scal_pool.tile([P, 1], f32, name=f"excum{r}")
                    with tc.tile_critical():
                        nc.vector.tensor_tensor(
                            out=nxt[:], in0=acc_r[:], in1=bufs[r][:], op=A.add
                        )._wait_ge(ex_sems[r], 2)
                    acc_r = nxt
                sb_s = scal_pool.tile([1, 1], f32)
                nc.gpsimd.tensor_reduce(
                    sb_s[:], acc_r[:], axis=mybir.AxisListType.C, op=A.add
                )
                s_sum = scal_pool.tile([P, 1], f32)
                nc.gpsimd.partition_broadcast(s_sum[:], sb_s[:])
            else:
                sb_s = scal_pool.tile([1, 1], f32)
                nc.gpsimd.tensor_reduce(
                    sb_s[:], red[:], axis=mybir.AxisListType.C, op=A.add
                )
                # ---- phase C: AllReduce the scalar across all cores ----
                cc_in = dram_pool.tile([1, 1], f32)
                cc_out = dram_pool.tile([1, 1], f32)
                nc.sync.dma_start(cc_in[:], sb_s[:])
                nc.gpsimd.collective_compute(
                    "AllReduce",
                    A.add,
                    replica_groups=[list(range(n_cores))],
                    ins=[cc_in[:]],
                    outs=[cc_out[:]],
                )
                cc_out_ap = cc_out[:]
                bcast_ap = bass.AP(
                    tensor=cc_out_ap.tensor,
                    offset=cc_out_ap.offset,
                    ap=[[0, P], [1, 1]],
                )
                s_sum = scal_pool.tile([P, 1], f32)
                nc.sync.dma_start(s_sum[:], bcast_ap)
            # thr = 0.5 * max(sum/N, EPS) = max(sum * (0.5/N), 0.5*EPS)
            # in ONE op — bit-identical (x0.5 is exact and commutes with
            # RNE rounding and max), and one fewer hop on the critical path.
            thr = scal_pool.tile([P, 1], f32)
            nc.vector.tensor_scalar(
                out=thr[:],
                in0=s_sum[:],
                scalar1=0.5 / float(w_elems_total),
                scalar2=0.5 * EPS,
                op0=A.mult,
                op1=A.max,
            )

            # ---- quantize helper: w^T strip [128i, OC] -> 2q in bf16 ----
            def quantize(c, ib):
                wst = win_pool.tile([P, OC], f32, tag="w", name=f"w_{c}_{ib}")
                nc.sync.dma_start(wst[:], wT[ts(ib, P), ds(c * OC, OC)])
                t2 = tq_pool.tile([P, OC], bf16, tag="t2", name=f"t2_{c}_{ib}")
                nc.vector.tensor_scalar(
                    out=t2[:],
                    in0=wst[:],
                    scalar1=thr[:],
                    scalar2=2.0,
                    op0=A.is_gt,
                    op1=A.mult,
                )
                s2 = sq_pool.tile([P, OC], bf16, tag="s2", name=f"s2_{c}_{ib}")
                nc.scalar.activation(
                    s2[:], wst[:], mybir.ActivationFunctionType.Sign, bias=thr[:]
                )
                q2 = qt_pool.tile(
                    [P, OC], bf16, tag=f"qt_{ib}_{c % 2}", name=f"qt_{c}_{ib}"
                )
                # q2 = (t2 - 1) + s2  in {-2, 0, 2}  (= 2q)
                nc.vector.scalar_tensor_tensor(
                    out=q2[:],
                    in0=t2[:],
                    scalar=-1.0,
                    in1=s2[:],
                    op0=A.add,
                    op1=A.add,
                )
                return q2

            psk = [0]  # rotating PSUM tag counter (8 banks)

            def evict(ps, c, tb):
                osb = out_pool.tile([P, OC], f32, tag="o")
                # psum holds x @ (2q)^T; scale by thr = s_w/2
                nc.scalar.activation(
                    osb[:], ps[:], mybir.ActivationFunctionType.Copy, scale=thr[:]
                )
                nc.sync.dma_start(out_sh[ts(tb, P), ds(c * OC, OC)], osb[:])

            def psum_tile(name):
                t = ps_acc.tile([P, OC], f32, tag=f"acc{psk[0] % 8}", name=name)
                psk[0] += 1
                return t

            # ---- chunk 0: i-block-major so matmuls start while x/w
            # stream in (PE never waits for the full first sweep).
            # Chunk 1 is quantized in the same pass.
            xt_tiles = [None] * n_ib
            qt_c = {}
            ps0 = [psum_tile(f"ps0_{tb}") for tb in range(n_tb)]
            for ib in range(n_ib):
                xb = xt_pool.tile([P, T], bf16, tag=f"xt_{ib}", name=f"xt_{ib}")
                nc.sync.dma_start(xb[:], xT[ts(ib, P), :])
                xt_tiles[ib] = xb
                qt_c[(0, ib)] = quantize(0, ib)
                for tb in range(n_tb):
                    nc.tensor.matmul(
                        ps0[tb][:],
                        lhsT=xt_tiles[ib][:, ts(tb, P)],
                        rhs=qt_c[(0, ib)][:],
                        start=(ib == 0),
                        stop=(ib == n_ib - 1),
                    )
            for tb in range(n_tb):
                evict(ps0[tb], 0, tb)

            # ---- remaining chunks: chunk 1 solo, then pairs sharing
            # one stationary block per (tb, ib).
            c = 1
            while c < n_oc:
                pair = [c] if (n_oc - c) % 2 == 1 else [c, c + 1]
                for cc in pair:
                    for ib in range(n_ib):
                        if (cc, ib) not in qt_c:
                            qt_c[(cc, ib)] = quantize(cc, ib)
                for tb in range(n_tb):
                    ps_tiles = [psum_tile(f"ps_{cc}_{tb}") for cc in pair]
                    for ib in range(n_ib):
                        lhs = xt_tiles[ib][:, ts(tb, P)]
                        for h, cc in enumerate(pair):
                            nc.tensor.matmul(
                                ps_tiles[h][:],
                                lhsT=lhs,
                                rhs=qt_c[(cc, ib)][:],
                                start=(ib == 0),
                                stop=(ib == n_ib - 1),
                            )
                    for h, cc in enumerate(pair):
                        evict(ps_tiles[h], cc, tb)
                for cc in pair:
                    for ib in range(n_ib):
                        del qt_c[(cc, ib)]
                c += len(pair)

    nc.compile()
    return nc


_CACHE = {}


def _get_nc(key):
    if key not in _CACHE:
        _CACHE[key] = build_nc(*key)
    return _CACHE[key]


def make_in_maps(x2d, weight, n_cores=N_CORES, tg=TG, fg=FG):
    """Host-side sharding: per-core pre-transposed inputs, x in bf16."""
    import ml_dtypes

    t_tot, i_full = x2d.shape
    o_full = weight.shape[0]
    t_sh = t_tot // tg
    o_sh = o_full // fg
    i_slab = i_full // tg
    x_bf = x2d.astype(ml_dtypes.bfloat16)
    wT_halves = {}
    for b in range(fg):
        wT_halves[b] = np.ascontiguousarray(weight[b * o_sh : (b + 1) * o_sh].T)
    in_maps = []
    for cid in range(n_cores):
        g, b = cid // fg, cid % fg
        # rotate i-rows of wT so rows [0, i_slab) are this core's slab;
        # the matmul contraction is a sum over i, invariant to the
        # rotation as long as xT rows are rotated identically.
        roll = -g * i_slab
        in_maps.append(
            {
                "xT": np.ascontiguousarray(
                    np.roll(x_bf[g * t_sh : (g + 1) * t_sh].T, roll, axis=0)
                ),
                "wT": np.roll(wT_halves[b], roll, axis=0),
            }
        )
    return in_maps


def run(x2d, weight, n_cores=N_CORES, tg=TG, fg=FG):
    """Run the sharded device computation: returns x @ q^T * s_w, [Ttot, O_full]."""
    from concourse.bass_utils import run_bass_kernel_spmd

    t_tot, i_full = x2d.shape
    o_full = weight.shape[0]
    t_sh = t_tot // tg
    o_sh = o_full // fg
    key = (t_sh, o_sh, i_full, n_cores, tg, o_full * i_full)
    nc = _get_nc(key)

    in_maps = make_in_maps(x2d, weight, n_cores, tg, fg)
    res = run_bass_kernel_spmd(nc, in_maps, core_ids=list(range(n_cores)))
    out = np.empty((t_tot, o_full), np.float32)
    for cid in range(n_cores):
        g, b = cid // fg, cid % fg
        out[g * t_sh : (g + 1) * t_sh, b * o_sh : (b + 1) * o_sh] = res.results[
            cid
        ]["out_sh"]
    return out


def kernel(x, weight, bias):
    x = np.asarray(x, np.float32)
    weight = np.asarray(weight, np.float32)
    bias = np.asarray(bias, np.float32)
    t_tot = x.shape[0] * x.shape[1]
    out = run(x.reshape(t_tot, x.shape[2]), weight)
    # bias term: out += bias * s_x (exact reference semantics; zero for
    # this problem's bias). The matmul term is s_x-invariant.
    if np.any(bias):
        s_x = np.float32(max(np.mean(np.abs(x)), EPS))
        out = out + (bias * s_x)[None, :]
    return out.reshape(x.shape[0], x.shape[1], weight.shape[0])
